# revision 53
# baseline (speedup 1.0000x reference)
"""Causal GQA self-attention (B=4,T=2048,C=2048,H=16,HKV=4,D=128) on 8 trn2 cores.

Sharding: core c -> (batch b = c//2, kv-group g = c%2). Each core computes the
attention output for its batch restricted to its 8 query heads (2 kv heads),
then the partial output projection against the matching 1024 rows of Wc.
Host sums the two partial outputs per batch and descales. No collectives.

Speed plan vs bf16 baseline:
- QKV projection and output projection run as fp8e4m3 DoubleRow matmuls with a
  3-term error-compensated split (a@b ~= a1@b1 + a2@b1 + a1@b2, fp32 PSUM),
  contracting 256 rows/instr at 0.5 cycles/row.
- Attention internals (q/k/v/p) are fp16. Softmax skips max subtraction
  (rmsnormed q,k bound |scores| <= sqrt(D)); exp is biased by e^-3 to keep
  fp16 headroom; the bias cancels in the normalization.
- rmsnorm scale (and the softmax 1/sqrt(D) for q) is folded into the
  [token -> d,t] transpose by multiplying with diag(rstd) on the PE.
- Causal structure exploited at 128-key-block granularity: scores / exp / PV
  / row-sum accumulation all narrowed to the valid query suffix; one shared
  [128,128] upper-triangular mask handles the diagonal blocks.
- Row sums via an all-ones [128,128] stationary matmul producing the sum
  replicated across partitions (no separate broadcast matmul / copy).
- Output projection is interleaved into the attention loop (t4-major) so its
  dense fp8 matmuls fill the PE gaps left by the exp/mask dependency chain.
- Weight DMAs are chunked per contraction pair in consumption order so the
  first projection matmul starts ~2us after launch.
"""

import math
import sys

import numpy as np

sys.path.insert(0, "/opt/trn_rl_repo")

import ml_dtypes

import concourse.bass as bass
import concourse.bass_isa as bass_isa
import concourse.mybir as mybir
import concourse.tile as tile
from concourse import bacc
from concourse.bass_utils import run_bass_kernel_spmd

F8 = mybir.dt.float8e4
F16 = mybir.dt.float16
FP32 = mybir.dt.float32
NPF8 = ml_dtypes.float8_e4m3
NPF16 = np.float16

B, T, C = 4, 2048, 2048
H, HKV, D = 16, 4, 128
LH, LKV = 8, 2           # local q heads / kv heads per core
NTT = T // 128           # token tiles
NCP = C // 256           # contraction pairs (256 rows per DoubleRow matmul)
NQT = T // 512           # query tiles of 512
NH = LH + LKV            # heads normed/roped per token tile
NCT = C // 128           # output row tiles (phase 3)
EPS = 1e-6
SM = 1.0 / math.sqrt(D)  # softmax scale (folded into q's rstd)
SW = 64.0                # Wq/Wk fp8 prescale
SV = 32.0                # Wv fp8 prescale (descaled via ones rowsum matmul)
SC = 64.0                # Wc fp8 prescale (descaled on host)
EXPB = -3.0              # exp bias, cancels in normalization

ACT = mybir.ActivationFunctionType
DR = mybir.MatmulPerfMode.DoubleRow

USE_GPSIMD_STATS = True   # square+reduce on Pool engine instead of DVE

LAST_RESULTS = None       # BassKernelResults of the most recent run (test.py)
_CACHED_NC = None


def _build_module():
    nc = bacc.Bacc("TRN2", target_bir_lowering=False, debug=False, num_devices=8)

    x1_d = nc.dram_tensor("x1", [NTT, 128, NCP, 2, 128], F8, kind="ExternalInput").ap()
    x2_d = nc.dram_tensor("x2", [NTT, 128, NCP, 2, 128], F8, kind="ExternalInput").ap()
    wq1_d = nc.dram_tensor("wq1", [128, NCP, 2, LH * D], F8, kind="ExternalInput").ap()
    wq2_d = nc.dram_tensor("wq2", [128, NCP, 2, LH * D], F8, kind="ExternalInput").ap()
    wkv1_d = nc.dram_tensor("wkv1", [128, NCP, 2, 512], F8, kind="ExternalInput").ap()
    wkv2_d = nc.dram_tensor("wkv2", [128, NCP, 2, 512], F8, kind="ExternalInput").ap()
    wc1_d = nc.dram_tensor("wc1", [128, LH // 2, 2, C], F8, kind="ExternalInput").ap()
    wc2_d = nc.dram_tensor("wc2", [128, LH // 2, 2, C], F8, kind="ExternalInput").ap()
    cs_d = nc.dram_tensor("cs", [NTT, 128, 128], F16, kind="ExternalInput").ap()
    tri_d = nc.dram_tensor("tri", [128, 128], F16, kind="ExternalInput").ap()
    id_d = nc.dram_tensor("ident", [128, 128], F16, kind="ExternalInput").ap()
    out_d = nc.dram_tensor("outT", [C, T], FP32, kind="ExternalOutput").ap()

    with tile.TileContext(nc) as tc:
        consts = tc.alloc_tile_pool(name="consts", bufs=1)
        persist = tc.alloc_tile_pool(name="persist", bufs=1)

        eye_s = consts.tile([128, 128], F16)
        nc.sync.dma_start(out=eye_s, in_=id_d)
        tri_s = consts.tile([128, 128], F16)
        nc.sync.dma_start(out=tri_s, in_=tri_d)
        ones_m = consts.tile([128, 128], F16)
        nc.vector.memset(ones_m, 1.0)
        bias_q = consts.tile([128, 1], FP32)
        nc.vector.memset(bias_q, SW * SW * EPS / (SM * SM))
        bias_k = consts.tile([128, 1], FP32)
        nc.vector.memset(bias_k, SW * SW * EPS)
        bias_e = consts.tile([128, 1], FP32)
        nc.vector.memset(bias_e, EXPB)

        # persistent activations
        qt_all = persist.tile([128, LH, T], F16)      # Q^T per head [d, t]
        kt_all = persist.tile([128, LKV, T], F16)     # K^T per kv head [d, t]
        v_all = persist.tile([128, NTT, LKV * D], F16)  # V per t-tile [t, d]
        y1_all = persist.tile([128, LH, T], F8)       # attn out y^T hi
        y2_all = persist.tile([128, LH, T], F8)       # attn out y^T lo residual

        # ---------------- phase 1: projections + rmsnorm + rope + transpose
        with (
            tc.tile_pool(name="wpool", bufs=1) as wpool,
            tc.tile_pool(name="xpool", bufs=4) as xpool,
            tc.tile_pool(name="cspool", bufs=4) as cspool,
            tc.tile_pool(name="stage", bufs=2) as stage,
            tc.tile_pool(name="dpool", bufs=3) as dpool,
            tc.tile_pool(name="stats", bufs=3) as stats,
            tc.tile_pool(name="ps1", bufs=2, space="PSUM") as ps1,
            tc.tile_pool(name="pstp", bufs=2, space="PSUM") as pstp,
        ):
            wq1_s = wpool.tile([128, NCP, 2, LH * D], F8)
            wkv1_s = wpool.tile([128, NCP, 2, 512], F8)
            wq2_s = wpool.tile([128, NCP, 2, LH * D], F8)
            wkv2_s = wpool.tile([128, NCP, 2, 512], F8)

            def fetch_x(tt):
                xt1 = xpool.tile([128, NCP, 2, 128], F8, tag="x1")
                nc.sync.dma_start(out=xt1, in_=x1_d[tt])
                xt2 = xpool.tile([128, NCP, 2, 128], F8, tag="x2")
                nc.sync.dma_start(out=xt2, in_=x2_d[tt])
                cs_s = cspool.tile([128, 2, 64], F16)
                nc.sync.dma_start(
                    out=cs_s, in_=cs_d[tt].rearrange("p (a b) -> p a b", a=2)
                )
                return xt1, xt2, cs_s

            # x tiles 0/1 queue ahead of the weight stream so the first
            # matmuls only wait for ctp=0's weight slices; 2/3 interleave
            # into the weight stream ahead of when their tiles run
            xfetch = [fetch_x(0), fetch_x(1)]
            for c in range(NCP):
                nc.sync.dma_start(out=wq1_s[:, c], in_=wq1_d[:, c])
                nc.sync.dma_start(out=wkv1_s[:, c], in_=wkv1_d[:, c])
                nc.sync.dma_start(out=wq2_s[:, c], in_=wq2_d[:, c])
                nc.sync.dma_start(out=wkv2_s[:, c], in_=wkv2_d[:, c])
                if c in (3, 5):
                    xfetch.append(fetch_x(2 + (c - 3) // 2))

            # transposes of tile t emit after tile t+1's projection matmuls
            # so the PE never waits on tile t's rope/rstd chain
            pending_tp = []

            def emit_tp():
                while pending_tp:
                    pending_tp.pop(0)()

            for tt in range(NTT):
                xt1, xt2, cs_s = xfetch.pop(0)

                qkv_ps = ps1.tile([128, 1536], FP32)
                terms = [(xt1, wq1_s, wkv1_s), (xt2, wq1_s, wkv1_s), (xt1, wq2_s, wkv2_s)]
                for ctp in range(NCP):
                    for ti, (xs, wq, wkv) in enumerate(terms):
                        st = ctp == 0 and ti == 0
                        sp = ctp == NCP - 1 and ti == len(terms) - 1
                        nc.tensor.matmul(
                            qkv_ps[:, 0:512], lhsT=xs[:, ctp],
                            rhs=wq[:, ctp, :, 0:512], start=st, stop=sp,
                            perf_mode=DR,
                        )
                        nc.tensor.matmul(
                            qkv_ps[:, 512:1024], lhsT=xs[:, ctp],
                            rhs=wq[:, ctp, :, 512:1024], start=st, stop=sp,
                            perf_mode=DR,
                        )
                        nc.tensor.matmul(
                            qkv_ps[:, 1024:1536], lhsT=xs[:, ctp],
                            rhs=wkv[:, ctp], start=st, stop=sp,
                            perf_mode=DR,
                        )

                emit_tp()

                # V (cols 1280:1536) straight out, fp16, still x SV
                nc.scalar.copy(v_all[:, tt], qkv_ps[:, 1280:1536])
                # q/k raw to fp16 SBUF (values x SW)
                raw = stage.tile([128, NH, 128], F16, tag="raw")
                nc.scalar.copy(
                    raw, qkv_ps[:, 0:1280].rearrange("p (h d) -> p h d", h=NH)
                )

                # rmsnorm stats: square on GPSIMD (idle), reduce on DVE.
                sq_eng = nc.gpsimd if USE_GPSIMD_STATS else nc.vector
                sq = stage.tile([128, NH, 128], FP32, tag="sq")
                sq_eng.tensor_mul(sq, raw, raw)
                ssq = stats.tile([128, NH], FP32)
                nc.vector.tensor_reduce(
                    out=ssq, in_=sq, axis=mybir.AxisListType.X, op=mybir.AluOpType.add
                )
                # rstd_q = SM / (SW*sqrt(ms+eps)); rstd_k = 1 / (SW*sqrt(ms+eps))
                srt = stats.tile([128, NH], FP32)
                nc.scalar.activation(
                    srt[:, 0:LH], ssq[:, 0:LH], ACT.Sqrt,
                    scale=1.0 / (D * SM * SM), bias=bias_q,
                )
                nc.scalar.activation(
                    srt[:, LH:NH], ssq[:, LH:NH], ACT.Sqrt,
                    scale=1.0 / D, bias=bias_k,
                )
                rstd = stats.tile([128, NH], FP32)
                nc.vector.reciprocal(out=rstd, in_=srt)

                # rope on raw (unscaled; rstd applied during transpose).
                # W columns are host-permuted even-dims-first so the rotation
                # halves are contiguous (keeps DVE 16-bit 2x mode)
                cos_b = cs_s[:, 0:1, :].broadcast_to([128, NH, 64])
                sin_b = cs_s[:, 1:2, :].broadcast_to([128, NH, 64])
                q_e, q_o = raw[:, :, 0:64], raw[:, :, 64:128]
                rp = stage.tile([128, NH, 128], F16, tag="rp")
                r_e, r_o = rp[:, :, 0:64], rp[:, :, 64:128]
                t1 = stage.tile([128, NH, 64], F16, tag="t1")
                t2 = stage.tile([128, NH, 64], F16, tag="t2")
                nc.vector.tensor_mul(t1, q_e, cos_b)
                nc.vector.tensor_mul(t2, q_o, sin_b)
                nc.vector.tensor_sub(r_e, t1, t2)
                t3 = stage.tile([128, NH, 64], F16, tag="t3")
                t4 = stage.tile([128, NH, 64], F16, tag="t4")
                nc.vector.tensor_mul(t3, q_e, sin_b)
                nc.vector.tensor_mul(t4, q_o, cos_b)
                nc.vector.tensor_add(r_o, t3, t4)

                # transpose to [d, t] with rstd folded in via diag matmul;
                # deferred so it emits after the next tile's projections
                def transpose_tile(tt=tt, rp=rp, rstd=rstd):
                    for g0, g1 in ((0, 4), (4, 8), (8, 10)):
                        n = g1 - g0
                        tp = pstp.tile([128, 512], FP32, tag="tp")
                        for i in range(g0, g1):
                            dg = dpool.tile([128, 128], F16)
                            nc.vector.tensor_scalar_mul(dg, eye_s, rstd[:, i:i + 1])
                            nc.tensor.matmul(
                                tp[:, (i - g0) * 128:(i - g0 + 1) * 128],
                                lhsT=rp[:, i], rhs=dg, start=True, stop=True,
                            )
                        view = tp[:, 0:n * 128].rearrange("p (h d) -> p h d", h=n)
                        if g0 < 8:
                            dst = qt_all[:, g0:g1, tt * 128:(tt + 1) * 128]
                        else:
                            dst = kt_all[:, g0 - 8:g1 - 8, tt * 128:(tt + 1) * 128]
                        nc.scalar.copy(dst, view)
                pending_tp.append(transpose_tile)

                if tt + 4 < NTT:
                    xfetch.append(fetch_x(tt + 4))

            emit_tp()

        # ---------------- phase 2: attention, with phase-3 chunks interleaved
        with tc.tile_pool(name="wcpool", bufs=1) as wcpool:
            # Wc (fp8 pair) loads overlap attention; used by phase-3 chunks
            wc1_s = wcpool.tile([128, LH // 2, 2, C], F8)
            nc.sync.dma_start(out=wc1_s, in_=wc1_d)
            wc2_s = wcpool.tile([128, LH // 2, 2, C], F8)
            nc.sync.dma_start(out=wc2_s, in_=wc2_d)

            with (
                tc.tile_pool(name="ppool", bufs=8) as ppool,
                tc.tile_pool(name="accp", bufs=2) as accp,
                tc.tile_pool(name="rcpool", bufs=2) as rcpool,
                tc.tile_pool(name="ypool", bufs=6) as ypool,
                tc.tile_pool(name="outst", bufs=3) as outst,
                tc.tile_pool(name="rspool", bufs=2) as rspool,
                tc.tile_pool(name="pss", bufs=2, space="PSUM") as pss,
                tc.tile_pool(name="pso", bufs=2, space="PSUM") as pso,
                tc.tile_pool(name="psop", bufs=2, space="PSUM") as psop,
            ):
                p3_terms = [(y1_all, wc1_s), (y2_all, wc1_s), (y1_all, wc2_s)]

                # finishers (psum drain + DMA) run one emission point later so
                # they never head-block the ACT/DVE queues behind a PE wait
                p3_finishers = []

                def emit_p3_chunk(t4, ct):
                    """One output-projection chunk: 12 DR matmuls; the psum
                    drain + output DMA are deferred to the next chunk."""
                    # y stores for this t4 must be emitted before reading y
                    while tail_finishers:
                        tail_finishers.pop(0)()
                    while p3_finishers:
                        p3_finishers.pop(0)()
                    op_ps = psop.tile([128, 512], FP32, tag="op")
                    for ti, (ya, wc) in enumerate(p3_terms):
                        for lhp in range(LH // 2):
                            st = ti == 0 and lhp == 0
                            sp = ti == len(p3_terms) - 1 and lhp == LH // 2 - 1
                            nc.tensor.matmul(
                                op_ps,
                                lhsT=wc[:, lhp, :, ct * 128:(ct + 1) * 128],
                                rhs=ya[:, 2 * lhp:2 * lhp + 2, t4 * 512:(t4 + 1) * 512],
                                start=st, stop=sp, perf_mode=DR,
                            )

                    def finish(t4=t4, ct=ct, op_ps=op_ps):
                        ost = outst.tile([128, 512], FP32)
                        if ct % 2 == 0:
                            nc.scalar.copy(ost, op_ps)
                        else:
                            nc.vector.tensor_copy(out=ost, in_=op_ps)
                        nc.sync.dma_start(
                            out=out_d[ct * 128:(ct + 1) * 128, t4 * 512:(t4 + 1) * 512],
                            in_=ost,
                        )
                    p3_finishers.append(finish)

                # pending phase-3 work: chunks for t4 become ready once all lh
                # of qt=t4 have run; trickled into later iterations' gaps.
                p3_pending = []
                done_lh = [0] * NQT

                # deferred per-iteration tail (rowsum..yt), emitted after the
                # next iteration's first score pair to keep the PE queue fed;
                # the y1/y2 stores defer one step further (not latency-bound)
                pending_tail = []
                tail_finishers = []

                def emit_tail():
                    while pending_tail:
                        fin = pending_tail.pop(0)()
                        tail_finishers.append(fin)
                    # y1/y2 stores lag a few iterations: they are only read
                    # by p3 chunks (which force-drain) so they can vacate the
                    # tight blocks entirely
                    while len(tail_finishers) > 3:
                        tail_finishers.pop(0)()

                # 3-way interleave of qt 3/1/0 balances PE vs ACT load and
                # hides each iteration's softmax-tail latency under the other
                # two; qt=2 runs last, its gaps filled by the 48 output-
                # projection chunks that became ready (t4 in {3,1,0})
                schedule = [(q, l) for l in range(LH) for q in (1, 0)]
                schedule += [(2, l) for l in range(LH)]
                schedule += [(3, l) for l in range(LH)]

                class It:
                    """Per-(query-tile, head) iteration state."""

                    def __init__(self, qt, lh):
                        self.qt, self.lh = qt, lh
                        self.lkv = lh // (H // HKV)
                        self.npair = 2 * qt + 2
                        self.nkb = 2 * self.npair
                        self.q0 = qt * 512
                        self.o_ps = None
                        self.pacc = None
                        self.pts = [None] * self.npair

                def emit_scores(it, pi):
                    qt = it.qt
                    s_ps = pss.tile([128, 1024], FP32)
                    pt = ppool.tile([128, 2, 512], F16)
                    for e in range(2):
                        kb = 2 * pi + e
                        j = kb - 4 * qt
                        vs = 128 * j if j > 0 else 0
                        nc.tensor.matmul(
                            s_ps[:, e * 512 + vs:(e + 1) * 512],
                            lhsT=kt_all[:, it.lkv, kb * 128:(kb + 1) * 128],
                            rhs=qt_all[:, it.lh, it.q0 + vs:it.q0 + 512],
                            start=True, stop=True,
                        )
                    if pi < 2 * qt:
                        # off-diagonal pair: one full-width exp
                        nc.scalar.activation(
                            pt.rearrange("p a b -> p (a b)"), s_ps, ACT.Exp,
                            bias=bias_e, scale=1.0,
                        )
                    else:
                        # diagonal pair: narrowed per-block exp + tri mask
                        for e in range(2):
                            kb = 2 * pi + e
                            j = kb - 4 * qt
                            vs = 128 * j if j > 0 else 0
                            nc.scalar.activation(
                                pt[:, e, vs:512],
                                s_ps[:, e * 512 + vs:(e + 1) * 512],
                                ACT.Exp, bias=bias_e, scale=1.0,
                            )
                            nc.vector.tensor_mul(
                                pt[:, e, vs:vs + 128], pt[:, e, vs:vs + 128],
                                tri_s,
                            )
                    it.pts[pi] = pt

                def emit_pv_acc(it, pi):
                    qt = it.qt
                    if pi == 0:
                        it.o_ps = pso.tile([128, 512], FP32)
                        it.pacc = accp.tile([128, 512], F16)
                    pt = it.pts[pi]
                    it.pts[pi] = None
                    for e in range(2):
                        kb = 2 * pi + e
                        j = kb - 4 * qt
                        vs = 128 * j if j > 0 else 0
                        nc.tensor.matmul(
                            it.o_ps[:, vs:512],
                            lhsT=v_all[:, kb, it.lkv * D:(it.lkv + 1) * D],
                            rhs=pt[:, e, vs:512],
                            start=(kb == 0), stop=(kb == it.nkb - 1),
                        )
                    # row-sum accumulator, both key blocks folded into one
                    # [128, 512] lane (valid-region ops only)
                    for e in range(2):
                        kb = 2 * pi + e
                        j = kb - 4 * qt
                        vs = 128 * j if j > 0 else 0
                        if pi == 0 and e == 0:
                            nc.vector.tensor_copy(out=it.pacc, in_=pt[:, 0])
                        elif vs == 0:
                            nc.vector.tensor_add(it.pacc, it.pacc, pt[:, e])
                        else:
                            nc.vector.tensor_add(
                                it.pacc[:, vs:512], it.pacc[:, vs:512],
                                pt[:, e, vs:512],
                            )

                def make_tail(it):
                    def tail():
                        # softmax denominator, summed over keys and replicated
                        # to all partitions. qt<=1 iterations run in a block
                        # where ACT/DVE/Pool are tight but the PE and the p3
                        # psum bank are idle -> all-ones matmul there; the
                        # rest use a Pool partition all-reduce.
                        if it.qt <= 1:
                            rs = psop.tile([128, 512], FP32, tag="op")
                            nc.tensor.matmul(
                                rs, lhsT=ones_m, rhs=it.pacc,
                                start=True, stop=True,
                            )
                        else:
                            rs = rspool.tile([128, 512], FP32, tag="rs")
                            nc.gpsimd.partition_all_reduce(
                                rs, it.pacc, channels=128,
                                reduce_op=bass_isa.ReduceOp.add,
                            )
                        bc_sb = rcpool.tile([128, 512], FP32)
                        with nc.allow_low_precision(
                            reason="softmax denominator reciprocal; fp32 data"
                        ):
                            nc.vector.reciprocal(out=bc_sb, in_=rs)
                        yt = ypool.tile([128, 512], F16)
                        nc.vector.tensor_mul(yt, it.o_ps, bc_sb)

                        def finish():
                            y1s = y1_all[:, it.lh, it.qt * 512:(it.qt + 1) * 512]
                            y2s = y2_all[:, it.lh, it.qt * 512:(it.qt + 1) * 512]
                            nc.gpsimd.tensor_copy(out=y1s, in_=yt)
                            nc.vector.tensor_sub(y2s, yt, y1s)
                        return finish
                    return tail

                # globally software-pipelined pair stream: the scores+exp of
                # pair k+2 emit before the PV of pair k, across iteration
                # boundaries, so the PE never heads an exp it just requested
                iters = [It(qt, lh) for qt, lh in schedule]
                stream = [(it, pi) for it in iters for pi in range(it.npair)]
                for k in range(min(2, len(stream))):
                    emit_scores(*stream[k])
                for k, (it, pi) in enumerate(stream):
                    if k + 2 < len(stream):
                        emit_scores(*stream[k + 2])
                    if pi == 0:
                        emit_tail()
                        for _ in range(2):
                            if p3_pending:
                                emit_p3_chunk(*p3_pending.pop(0))
                    emit_pv_acc(it, pi)
                    if pi == it.npair - 1:
                        pending_tail.append(make_tail(it))
                        done_lh[it.qt] += 1
                        if done_lh[it.qt] == LH:
                            # all heads of this qt done -> p3 tile ready
                            p3_pending.extend((it.qt, ct) for ct in range(NCT))
                    elif pi < 2 and p3_pending:
                        emit_p3_chunk(*p3_pending.pop(0))

                emit_tail()
                while tail_finishers:
                    tail_finishers.pop(0)()
                for t4, ct in p3_pending:
                    emit_p3_chunk(t4, ct)
                while p3_finishers:
                    p3_finishers.pop(0)()

        persist.release()
        consts.release()

    nc.compile()
    return nc


def _q8pair(a):
    """fp8 hi/lo split of a float32 array."""
    a1 = a.astype(NPF8)
    a2 = (a - a1.astype(np.float32)).astype(NPF8)
    return a1, a2


def _prep_inputs(x, freqs_cis, Wq, Wk, Wv, Wc):
    """Host-side shard + layout + quantization prep; returns 8 input maps."""
    x = np.asarray(x, dtype=np.float32)
    freqs_cis = np.asarray(freqs_cis, dtype=np.float32)

    x1, x2 = _q8pair(x)

    def arr_x(a):  # [T, C] -> [NTT, 128(c-part), NCP, 2, 128(tok)]
        t = a.T.reshape(NCP, 2, 128, NTT, 128).transpose(3, 2, 0, 1, 4)
        return np.ascontiguousarray(t)

    x1s = [arr_x(x1[b]) for b in range(B)]
    x2s = [arr_x(x2[b]) for b in range(B)]

    cs = np.concatenate([freqs_cis[:, :, 0], freqs_cis[:, :, 1]], axis=1)  # [T,128]
    cs = np.ascontiguousarray(cs.reshape(NTT, 128, 128)).astype(NPF16)

    # upper-triangular inclusive mask: tri[r, j] = 1 if j >= r
    tri = np.triu(np.ones((128, 128), dtype=NPF16))

    ident = np.eye(128, dtype=NPF16)

    def arr_w(a):  # [C, n] -> [128, NCP, 2, n]
        return np.ascontiguousarray(a.reshape(NCP, 2, 128, a.shape[1]).transpose(2, 0, 1, 3))

    # permute each head's D dims even-first/odd-last for contiguous rope;
    # scores are invariant since q and k share the permutation
    dperm = np.concatenate([np.arange(0, D, 2), np.arange(1, D, 2)])

    def permute_heads(w, nh):
        return np.ascontiguousarray(
            w.reshape(C, nh, D)[:, :, dperm].reshape(C, nh * D)
        )

    in_maps = []
    for core in range(8):
        b, g = divmod(core, 2)
        wqf = permute_heads(
            np.float32(Wq[:, g * LH * D:(g + 1) * LH * D]), LH
        ) * SW
        wq1, wq2 = _q8pair(wqf)
        wkvf = np.concatenate(
            [
                permute_heads(
                    np.float32(Wk[:, g * LKV * D:(g + 1) * LKV * D]), LKV
                ) * SW,
                np.float32(Wv[:, g * LKV * D:(g + 1) * LKV * D]) * SV,
            ],
            axis=1,
        )
        wkv1, wkv2 = _q8pair(wkvf)
        wcf = np.float32(Wc[g * LH * D:(g + 1) * LH * D]) * SC  # [1024, C]
        wc1, wc2 = _q8pair(wcf)

        def arr_wc(a):  # [1024, C] -> [128(d), 4(lhp), 2, C]
            return np.ascontiguousarray(a.reshape(LH // 2, 2, 128, C).transpose(2, 0, 1, 3))

        in_maps.append(
            {
                "x1": x1s[b],
                "x2": x2s[b],
                "wq1": arr_w(wq1),
                "wq2": arr_w(wq2),
                "wkv1": arr_w(wkv1),
                "wkv2": arr_w(wkv2),
                "wc1": arr_wc(wc1),
                "wc2": arr_wc(wc2),
                "cs": cs,
                "tri": tri,
                "ident": ident,
            }
        )
    return in_maps


def kernel(x, freqs_cis, Wq, Wk, Wv, Wc):
    global LAST_RESULTS, _CACHED_NC
    if _CACHED_NC is None:
        _CACHED_NC = _build_module()
    nc = _CACHED_NC
    in_maps = _prep_inputs(x, freqs_cis, Wq, Wk, Wv, Wc)
    res = run_bass_kernel_spmd(nc, in_maps, core_ids=list(range(8)))
    LAST_RESULTS = res
    out = np.empty((B, T, C), dtype=np.float32)
    for b in range(B):
        acc = res.results[2 * b]["outT"] + res.results[2 * b + 1]["outT"]
        # SV: v-projection prescale, cancelled here instead of on-device
        out[b] = acc.T / (SC * SV)
    return out


# revision 54
# speedup vs baseline: 1.0346x; 1.0346x over previous
"""Causal GQA self-attention (B=4,T=2048,C=2048,H=16,HKV=4,D=128) on 8 trn2 cores.

Sharding: core c -> (batch b = c//2, kv-group g = c%2). Each core computes the
attention output for its batch restricted to its 8 query heads (2 kv heads),
then the partial output projection against the matching 1024 rows of Wc.
Host sums the two partial outputs per batch and descales. No collectives.

Speed plan vs bf16 baseline:
- QKV projection and output projection run as fp8e4m3 DoubleRow matmuls with a
  3-term error-compensated split (a@b ~= a1@b1 + a2@b1 + a1@b2, fp32 PSUM),
  contracting 256 rows/instr at 0.5 cycles/row.
- Attention internals (q/k/v/p) are fp16. Softmax skips max subtraction
  (rmsnormed q,k bound |scores| <= sqrt(D)); exp is biased by e^-3 to keep
  fp16 headroom; the bias cancels in the normalization.
- rmsnorm scale (and the softmax 1/sqrt(D) for q) is folded into the
  [token -> d,t] transpose by multiplying with diag(rstd) on the PE.
- Causal structure exploited at 128-key-block granularity: scores / exp / PV
  / row-sum accumulation all narrowed to the valid query suffix; one shared
  [128,128] upper-triangular mask handles the diagonal blocks.
- Row sums via an all-ones [128,128] stationary matmul producing the sum
  replicated across partitions (no separate broadcast matmul / copy).
- Output projection is interleaved into the attention loop (t4-major) so its
  dense fp8 matmuls fill the PE gaps left by the exp/mask dependency chain.
- Weight DMAs are chunked per contraction pair in consumption order so the
  first projection matmul starts ~2us after launch.
"""

import math
import sys

import numpy as np

sys.path.insert(0, "/opt/trn_rl_repo")

import ml_dtypes

import concourse.bass as bass
import concourse.bass_isa as bass_isa
import concourse.mybir as mybir
import concourse.tile as tile
from concourse import bacc
from concourse.bass_utils import run_bass_kernel_spmd

F8 = mybir.dt.float8e4
F16 = mybir.dt.float16
FP32 = mybir.dt.float32
NPF8 = ml_dtypes.float8_e4m3
NPF16 = np.float16

B, T, C = 4, 2048, 2048
H, HKV, D = 16, 4, 128
LH, LKV = 8, 2           # local q heads / kv heads per core
NTT = T // 128           # token tiles
NCP = C // 256           # contraction pairs (256 rows per DoubleRow matmul)
NQT = T // 512           # query tiles of 512
NH = LH + LKV            # heads normed/roped per token tile
NCT = C // 128           # output row tiles (phase 3)
EPS = 1e-6
SM = 1.0 / math.sqrt(D)  # softmax scale (folded into q's rstd)
SW = 64.0                # Wq/Wk fp8 prescale
SV = 32.0                # Wv fp8 prescale (descaled via ones rowsum matmul)
SC = 64.0                # Wc fp8 prescale (descaled on host)
EXPB = -3.0              # exp bias, cancels in normalization

ACT = mybir.ActivationFunctionType
DR = mybir.MatmulPerfMode.DoubleRow

USE_GPSIMD_STATS = False  # square on Pool engine instead of DVE

LAST_RESULTS = None       # BassKernelResults of the most recent run (test.py)
_CACHED_NC = None


def _build_module():
    nc = bacc.Bacc("TRN2", target_bir_lowering=False, debug=False, num_devices=8)

    x1_d = nc.dram_tensor("x1", [NTT, 128, NCP, 2, 128], F8, kind="ExternalInput").ap()
    x2_d = nc.dram_tensor("x2", [NTT, 128, NCP, 2, 128], F8, kind="ExternalInput").ap()
    wq1_d = nc.dram_tensor("wq1", [128, NCP, 2, LH * D], F8, kind="ExternalInput").ap()
    wq2_d = nc.dram_tensor("wq2", [128, NCP, 2, LH * D], F8, kind="ExternalInput").ap()
    wkv1_d = nc.dram_tensor("wkv1", [128, NCP, 2, 512], F8, kind="ExternalInput").ap()
    wkv2_d = nc.dram_tensor("wkv2", [128, NCP, 2, 512], F8, kind="ExternalInput").ap()
    wc1_d = nc.dram_tensor("wc1", [128, LH // 2, 2, C], F8, kind="ExternalInput").ap()
    wc2_d = nc.dram_tensor("wc2", [128, LH // 2, 2, C], F8, kind="ExternalInput").ap()
    cs_d = nc.dram_tensor("cs", [NTT, 128, 128], F16, kind="ExternalInput").ap()
    tri_d = nc.dram_tensor("tri", [128, 128], F16, kind="ExternalInput").ap()
    id_d = nc.dram_tensor("ident", [128, 128], F16, kind="ExternalInput").ap()
    out_d = nc.dram_tensor("outT", [C, T], FP32, kind="ExternalOutput").ap()

    with tile.TileContext(nc) as tc:
        consts = tc.alloc_tile_pool(name="consts", bufs=1)
        persist = tc.alloc_tile_pool(name="persist", bufs=1)

        eye_s = consts.tile([128, 128], F16)
        nc.sync.dma_start(out=eye_s, in_=id_d)
        tri_s = consts.tile([128, 128], F16)
        nc.sync.dma_start(out=tri_s, in_=tri_d)
        ones_m = consts.tile([128, 128], F16)
        nc.vector.memset(ones_m, 1.0)
        bias_q = consts.tile([128, 1], FP32)
        nc.vector.memset(bias_q, SW * SW * EPS / (SM * SM))
        bias_k = consts.tile([128, 1], FP32)
        nc.vector.memset(bias_k, SW * SW * EPS)
        bias_e = consts.tile([128, 1], FP32)
        nc.vector.memset(bias_e, EXPB)

        # persistent activations
        qt_all = persist.tile([128, LH, T], F16)      # Q^T per head [d, t]
        kt_all = persist.tile([128, LKV, T], F16)     # K^T per kv head [d, t]
        v_all = persist.tile([128, NTT, LKV * D], F16)  # V per t-tile [t, d]
        y1_all = persist.tile([128, LH, T], F8)       # attn out y^T hi
        y2_all = persist.tile([128, LH, T], F8)       # attn out y^T lo residual

        # ---------------- phase 1: projections + rmsnorm + rope + transpose
        with (
            tc.tile_pool(name="wpool", bufs=1) as wpool,
            tc.tile_pool(name="xpool", bufs=4) as xpool,
            tc.tile_pool(name="cspool", bufs=4) as cspool,
            tc.tile_pool(name="stage", bufs=2) as stage,
            tc.tile_pool(name="dpool", bufs=3) as dpool,
            tc.tile_pool(name="stats", bufs=3) as stats,
            tc.tile_pool(name="ps1", bufs=2, space="PSUM") as ps1,
            tc.tile_pool(name="pstp", bufs=2, space="PSUM") as pstp,
        ):
            wq1_s = wpool.tile([128, NCP, 2, LH * D], F8)
            wkv1_s = wpool.tile([128, NCP, 2, 512], F8)
            wq2_s = wpool.tile([128, NCP, 2, LH * D], F8)
            wkv2_s = wpool.tile([128, NCP, 2, 512], F8)

            def fetch_x(tt):
                xt1 = xpool.tile([128, NCP, 2, 128], F8, tag="x1")
                nc.sync.dma_start(out=xt1, in_=x1_d[tt])
                xt2 = xpool.tile([128, NCP, 2, 128], F8, tag="x2")
                nc.sync.dma_start(out=xt2, in_=x2_d[tt])
                cs_s = cspool.tile([128, 2, 64], F16)
                nc.sync.dma_start(
                    out=cs_s, in_=cs_d[tt].rearrange("p (a b) -> p a b", a=2)
                )
                return xt1, xt2, cs_s

            # x tiles 0/1 queue ahead of the weight stream so the first
            # matmuls only wait for ctp=0's weight slices; 2/3 interleave
            # into the weight stream ahead of when their tiles run
            xfetch = [fetch_x(0), fetch_x(1)]
            for c in range(NCP):
                nc.sync.dma_start(out=wq1_s[:, c], in_=wq1_d[:, c])
                nc.sync.dma_start(out=wkv1_s[:, c], in_=wkv1_d[:, c])
                nc.sync.dma_start(out=wq2_s[:, c], in_=wq2_d[:, c])
                nc.sync.dma_start(out=wkv2_s[:, c], in_=wkv2_d[:, c])
                if c in (3, 5):
                    xfetch.append(fetch_x(2 + (c - 3) // 2))

            # transposes of tile t emit after tile t+1's projection matmuls
            # so the PE never waits on tile t's rope/rstd chain
            pending_tp = []

            def emit_tp():
                while pending_tp:
                    pending_tp.pop(0)()

            for tt in range(NTT):
                xt1, xt2, cs_s = xfetch.pop(0)

                qkv_ps = ps1.tile([128, 1536], FP32)
                terms = [(xt1, wq1_s, wkv1_s), (xt2, wq1_s, wkv1_s), (xt1, wq2_s, wkv2_s)]
                for ctp in range(NCP):
                    for ti, (xs, wq, wkv) in enumerate(terms):
                        st = ctp == 0 and ti == 0
                        sp = ctp == NCP - 1 and ti == len(terms) - 1
                        nc.tensor.matmul(
                            qkv_ps[:, 0:512], lhsT=xs[:, ctp],
                            rhs=wq[:, ctp, :, 0:512], start=st, stop=sp,
                            perf_mode=DR,
                        )
                        nc.tensor.matmul(
                            qkv_ps[:, 512:1024], lhsT=xs[:, ctp],
                            rhs=wq[:, ctp, :, 512:1024], start=st, stop=sp,
                            perf_mode=DR,
                        )
                        nc.tensor.matmul(
                            qkv_ps[:, 1024:1536], lhsT=xs[:, ctp],
                            rhs=wkv[:, ctp], start=st, stop=sp,
                            perf_mode=DR,
                        )

                emit_tp()

                # V (cols 1280:1536) straight out, fp16, still x SV
                nc.scalar.copy(v_all[:, tt], qkv_ps[:, 1280:1536])
                # q/k raw to fp16 SBUF (values x SW)
                raw = stage.tile([128, NH, 128], F16, tag="raw")
                nc.scalar.copy(
                    raw, qkv_ps[:, 0:1280].rearrange("p (h d) -> p h d", h=NH)
                )

                # rmsnorm stats: square on GPSIMD (idle), reduce on DVE.
                sq_eng = nc.gpsimd if USE_GPSIMD_STATS else nc.vector
                sq = stage.tile([128, NH, 128], FP32, tag="sq")
                sq_eng.tensor_mul(sq, raw, raw)
                ssq = stats.tile([128, NH], FP32)
                nc.vector.tensor_reduce(
                    out=ssq, in_=sq, axis=mybir.AxisListType.X, op=mybir.AluOpType.add
                )
                # rstd_q = SM / (SW*sqrt(ms+eps)); rstd_k = 1 / (SW*sqrt(ms+eps))
                srt = stats.tile([128, NH], FP32)
                nc.scalar.activation(
                    srt[:, 0:LH], ssq[:, 0:LH], ACT.Sqrt,
                    scale=1.0 / (D * SM * SM), bias=bias_q,
                )
                nc.scalar.activation(
                    srt[:, LH:NH], ssq[:, LH:NH], ACT.Sqrt,
                    scale=1.0 / D, bias=bias_k,
                )
                rstd = stats.tile([128, NH], FP32)
                nc.vector.reciprocal(out=rstd, in_=srt)

                # rope on raw (unscaled; rstd applied during transpose).
                # W columns are host-permuted even-dims-first so the rotation
                # halves are contiguous (keeps DVE 16-bit 2x mode)
                cos_b = cs_s[:, 0:1, :].broadcast_to([128, NH, 64])
                sin_b = cs_s[:, 1:2, :].broadcast_to([128, NH, 64])
                q_e, q_o = raw[:, :, 0:64], raw[:, :, 64:128]
                rp = stage.tile([128, NH, 128], F16, tag="rp")
                r_e, r_o = rp[:, :, 0:64], rp[:, :, 64:128]
                t1 = stage.tile([128, NH, 64], F16, tag="t1")
                t2 = stage.tile([128, NH, 64], F16, tag="t2")
                nc.vector.tensor_mul(t1, q_e, cos_b)
                nc.vector.tensor_mul(t2, q_o, sin_b)
                nc.vector.tensor_sub(r_e, t1, t2)
                t3 = stage.tile([128, NH, 64], F16, tag="t3")
                t4 = stage.tile([128, NH, 64], F16, tag="t4")
                nc.vector.tensor_mul(t3, q_e, sin_b)
                nc.vector.tensor_mul(t4, q_o, cos_b)
                nc.vector.tensor_add(r_o, t3, t4)

                # transpose to [d, t] with rstd folded in via diag matmul;
                # deferred so it emits after the next tile's projections
                def transpose_tile(tt=tt, rp=rp, rstd=rstd):
                    for g0, g1 in ((0, 4), (4, 8), (8, 10)):
                        n = g1 - g0
                        tp = pstp.tile([128, 512], FP32, tag="tp")
                        for i in range(g0, g1):
                            dg = dpool.tile([128, 128], F16)
                            nc.vector.tensor_scalar_mul(dg, eye_s, rstd[:, i:i + 1])
                            nc.tensor.matmul(
                                tp[:, (i - g0) * 128:(i - g0 + 1) * 128],
                                lhsT=rp[:, i], rhs=dg, start=True, stop=True,
                            )
                        view = tp[:, 0:n * 128].rearrange("p (h d) -> p h d", h=n)
                        if g0 < 8:
                            dst = qt_all[:, g0:g1, tt * 128:(tt + 1) * 128]
                        else:
                            dst = kt_all[:, g0 - 8:g1 - 8, tt * 128:(tt + 1) * 128]
                        nc.scalar.copy(dst, view)
                pending_tp.append(transpose_tile)

                if tt + 4 < NTT:
                    xfetch.append(fetch_x(tt + 4))

            emit_tp()

        # ---------------- phase 2: attention, with phase-3 chunks interleaved
        with tc.tile_pool(name="wcpool", bufs=1) as wcpool:
            # Wc (fp8 pair) loads overlap attention; used by phase-3 chunks
            wc1_s = wcpool.tile([128, LH // 2, 2, C], F8)
            nc.sync.dma_start(out=wc1_s, in_=wc1_d)
            wc2_s = wcpool.tile([128, LH // 2, 2, C], F8)
            nc.sync.dma_start(out=wc2_s, in_=wc2_d)

            with (
                tc.tile_pool(name="ppool", bufs=8) as ppool,
                tc.tile_pool(name="accp", bufs=2) as accp,
                tc.tile_pool(name="rcpool", bufs=2) as rcpool,
                tc.tile_pool(name="ypool", bufs=6) as ypool,
                tc.tile_pool(name="outst", bufs=3) as outst,
                tc.tile_pool(name="rspool", bufs=2) as rspool,
                tc.tile_pool(name="pss", bufs=2, space="PSUM") as pss,
                tc.tile_pool(name="pso", bufs=2, space="PSUM") as pso,
                tc.tile_pool(name="psop", bufs=2, space="PSUM") as psop,
            ):
                p3_terms = [(y1_all, wc1_s), (y2_all, wc1_s), (y1_all, wc2_s)]

                # finishers (psum drain + DMA) run one emission point later so
                # they never head-block the ACT/DVE queues behind a PE wait
                p3_finishers = []

                def emit_p3_chunk(t4, ct):
                    """One output-projection chunk: 12 DR matmuls; the psum
                    drain + output DMA are deferred to the next chunk."""
                    # y stores for this t4 must be emitted before reading y
                    while tail_finishers:
                        tail_finishers.pop(0)()
                    while p3_finishers:
                        p3_finishers.pop(0)()
                    op_ps = psop.tile([128, 512], FP32, tag="op")
                    for ti, (ya, wc) in enumerate(p3_terms):
                        for lhp in range(LH // 2):
                            st = ti == 0 and lhp == 0
                            sp = ti == len(p3_terms) - 1 and lhp == LH // 2 - 1
                            nc.tensor.matmul(
                                op_ps,
                                lhsT=wc[:, lhp, :, ct * 128:(ct + 1) * 128],
                                rhs=ya[:, 2 * lhp:2 * lhp + 2, t4 * 512:(t4 + 1) * 512],
                                start=st, stop=sp, perf_mode=DR,
                            )

                    def finish(t4=t4, ct=ct, op_ps=op_ps):
                        ost = outst.tile([128, 512], FP32)
                        if ct % 2 == 0:
                            nc.scalar.copy(ost, op_ps)
                        else:
                            nc.vector.tensor_copy(out=ost, in_=op_ps)
                        nc.sync.dma_start(
                            out=out_d[ct * 128:(ct + 1) * 128, t4 * 512:(t4 + 1) * 512],
                            in_=ost,
                        )
                    p3_finishers.append(finish)

                # pending phase-3 work: chunks for t4 become ready once all lh
                # of qt=t4 have run; trickled into later iterations' gaps.
                p3_pending = []
                done_lh = [0] * NQT

                # deferred per-iteration tail (rowsum..yt), emitted after the
                # next iteration's first score pair to keep the PE queue fed;
                # the y1/y2 stores defer one step further (not latency-bound)
                pending_tail = []
                tail_finishers = []

                def emit_tail():
                    while pending_tail:
                        fin = pending_tail.pop(0)()
                        tail_finishers.append(fin)
                    # y1/y2 stores lag a few iterations: they are only read
                    # by p3 chunks (which force-drain) so they can vacate the
                    # tight blocks entirely
                    while len(tail_finishers) > 3:
                        tail_finishers.pop(0)()

                # 3-way interleave of qt 3/1/0 balances PE vs ACT load and
                # hides each iteration's softmax-tail latency under the other
                # two; qt=2 runs last, its gaps filled by the 48 output-
                # projection chunks that became ready (t4 in {3,1,0})
                schedule = [(q, l) for l in range(LH) for q in (1, 0)]
                schedule += [(2, l) for l in range(LH)]
                schedule += [(3, l) for l in range(LH)]

                class It:
                    """Per-(query-tile, head) iteration state."""

                    def __init__(self, qt, lh):
                        self.qt, self.lh = qt, lh
                        self.lkv = lh // (H // HKV)
                        self.npair = 2 * qt + 2
                        self.nkb = 2 * self.npair
                        self.q0 = qt * 512
                        self.o_ps = None
                        self.pacc = None
                        self.pts = [None] * self.npair

                def emit_scores(it, pi):
                    qt = it.qt
                    s_ps = pss.tile([128, 1024], FP32)
                    pt = ppool.tile([128, 2, 512], F16)
                    for e in range(2):
                        kb = 2 * pi + e
                        j = kb - 4 * qt
                        vs = 128 * j if j > 0 else 0
                        nc.tensor.matmul(
                            s_ps[:, e * 512 + vs:(e + 1) * 512],
                            lhsT=kt_all[:, it.lkv, kb * 128:(kb + 1) * 128],
                            rhs=qt_all[:, it.lh, it.q0 + vs:it.q0 + 512],
                            start=True, stop=True,
                        )
                    if pi < 2 * qt:
                        # off-diagonal pair: one full-width exp
                        nc.scalar.activation(
                            pt.rearrange("p a b -> p (a b)"), s_ps, ACT.Exp,
                            bias=bias_e, scale=1.0,
                        )
                    else:
                        # diagonal pair: narrowed per-block exp + tri mask
                        for e in range(2):
                            kb = 2 * pi + e
                            j = kb - 4 * qt
                            vs = 128 * j if j > 0 else 0
                            nc.scalar.activation(
                                pt[:, e, vs:512],
                                s_ps[:, e * 512 + vs:(e + 1) * 512],
                                ACT.Exp, bias=bias_e, scale=1.0,
                            )
                            nc.vector.tensor_mul(
                                pt[:, e, vs:vs + 128], pt[:, e, vs:vs + 128],
                                tri_s,
                            )
                    it.pts[pi] = pt

                def emit_pv_acc(it, pi):
                    qt = it.qt
                    if pi == 0:
                        it.o_ps = pso.tile([128, 512], FP32)
                        it.pacc = accp.tile([128, 512], F16)
                    pt = it.pts[pi]
                    it.pts[pi] = None
                    for e in range(2):
                        kb = 2 * pi + e
                        j = kb - 4 * qt
                        vs = 128 * j if j > 0 else 0
                        nc.tensor.matmul(
                            it.o_ps[:, vs:512],
                            lhsT=v_all[:, kb, it.lkv * D:(it.lkv + 1) * D],
                            rhs=pt[:, e, vs:512],
                            start=(kb == 0), stop=(kb == it.nkb - 1),
                        )
                    # row-sum accumulator, both key blocks folded into one
                    # [128, 512] lane (valid-region ops only)
                    for e in range(2):
                        kb = 2 * pi + e
                        j = kb - 4 * qt
                        vs = 128 * j if j > 0 else 0
                        if pi == 0 and e == 0:
                            nc.vector.tensor_copy(out=it.pacc, in_=pt[:, 0])
                        elif vs == 0:
                            nc.vector.tensor_add(it.pacc, it.pacc, pt[:, e])
                        else:
                            nc.vector.tensor_add(
                                it.pacc[:, vs:512], it.pacc[:, vs:512],
                                pt[:, e, vs:512],
                            )

                def make_tail(it):
                    def tail():
                        # softmax denominator, summed over keys and replicated
                        # to all partitions. qt<=1 iterations run in a block
                        # where ACT/DVE/Pool are tight but the PE and the p3
                        # psum bank are idle -> all-ones matmul there; the
                        # rest use a Pool partition all-reduce.
                        if it.qt <= 1:
                            rs = psop.tile([128, 512], FP32, tag="op")
                            nc.tensor.matmul(
                                rs, lhsT=ones_m, rhs=it.pacc,
                                start=True, stop=True,
                            )
                        else:
                            rs = rspool.tile([128, 512], FP32, tag="rs")
                            nc.gpsimd.partition_all_reduce(
                                rs, it.pacc, channels=128,
                                reduce_op=bass_isa.ReduceOp.add,
                            )
                        bc_sb = rcpool.tile([128, 512], FP32)
                        with nc.allow_low_precision(
                            reason="softmax denominator reciprocal; fp32 data"
                        ):
                            nc.vector.reciprocal(out=bc_sb, in_=rs)
                        yt = ypool.tile([128, 512], F16)
                        nc.vector.tensor_mul(yt, it.o_ps, bc_sb)

                        def finish():
                            y1s = y1_all[:, it.lh, it.qt * 512:(it.qt + 1) * 512]
                            y2s = y2_all[:, it.lh, it.qt * 512:(it.qt + 1) * 512]
                            nc.gpsimd.tensor_copy(out=y1s, in_=yt)
                            nc.vector.tensor_sub(y2s, yt, y1s)
                        return finish
                    return tail

                # globally software-pipelined pair stream: the scores+exp of
                # pair k+2 emit before the PV of pair k, across iteration
                # boundaries, so the PE never heads an exp it just requested
                iters = [It(qt, lh) for qt, lh in schedule]
                stream = [(it, pi) for it in iters for pi in range(it.npair)]
                for k in range(min(2, len(stream))):
                    emit_scores(*stream[k])
                for k, (it, pi) in enumerate(stream):
                    if k + 2 < len(stream):
                        emit_scores(*stream[k + 2])
                    if pi == 0:
                        emit_tail()
                        for _ in range(2):
                            if p3_pending:
                                emit_p3_chunk(*p3_pending.pop(0))
                    emit_pv_acc(it, pi)
                    if pi == it.npair - 1:
                        pending_tail.append(make_tail(it))
                        done_lh[it.qt] += 1
                        if done_lh[it.qt] == LH:
                            # all heads of this qt done -> p3 tile ready
                            p3_pending.extend((it.qt, ct) for ct in range(NCT))
                    elif pi < 2 and p3_pending:
                        emit_p3_chunk(*p3_pending.pop(0))

                emit_tail()
                while tail_finishers:
                    tail_finishers.pop(0)()
                for t4, ct in p3_pending:
                    emit_p3_chunk(t4, ct)
                while p3_finishers:
                    p3_finishers.pop(0)()

        persist.release()
        consts.release()

    nc.compile()
    return nc


def _q8pair(a):
    """fp8 hi/lo split of a float32 array."""
    a1 = a.astype(NPF8)
    a2 = (a - a1.astype(np.float32)).astype(NPF8)
    return a1, a2


def _prep_inputs(x, freqs_cis, Wq, Wk, Wv, Wc):
    """Host-side shard + layout + quantization prep; returns 8 input maps."""
    x = np.asarray(x, dtype=np.float32)
    freqs_cis = np.asarray(freqs_cis, dtype=np.float32)

    x1, x2 = _q8pair(x)

    def arr_x(a):  # [T, C] -> [NTT, 128(c-part), NCP, 2, 128(tok)]
        t = a.T.reshape(NCP, 2, 128, NTT, 128).transpose(3, 2, 0, 1, 4)
        return np.ascontiguousarray(t)

    x1s = [arr_x(x1[b]) for b in range(B)]
    x2s = [arr_x(x2[b]) for b in range(B)]

    cs = np.concatenate([freqs_cis[:, :, 0], freqs_cis[:, :, 1]], axis=1)  # [T,128]
    cs = np.ascontiguousarray(cs.reshape(NTT, 128, 128)).astype(NPF16)

    # upper-triangular inclusive mask: tri[r, j] = 1 if j >= r
    tri = np.triu(np.ones((128, 128), dtype=NPF16))

    ident = np.eye(128, dtype=NPF16)

    def arr_w(a):  # [C, n] -> [128, NCP, 2, n]
        return np.ascontiguousarray(a.reshape(NCP, 2, 128, a.shape[1]).transpose(2, 0, 1, 3))

    # permute each head's D dims even-first/odd-last for contiguous rope;
    # scores are invariant since q and k share the permutation
    dperm = np.concatenate([np.arange(0, D, 2), np.arange(1, D, 2)])

    def permute_heads(w, nh):
        return np.ascontiguousarray(
            w.reshape(C, nh, D)[:, :, dperm].reshape(C, nh * D)
        )

    in_maps = []
    for core in range(8):
        b, g = divmod(core, 2)
        wqf = permute_heads(
            np.float32(Wq[:, g * LH * D:(g + 1) * LH * D]), LH
        ) * SW
        wq1, wq2 = _q8pair(wqf)
        wkvf = np.concatenate(
            [
                permute_heads(
                    np.float32(Wk[:, g * LKV * D:(g + 1) * LKV * D]), LKV
                ) * SW,
                np.float32(Wv[:, g * LKV * D:(g + 1) * LKV * D]) * SV,
            ],
            axis=1,
        )
        wkv1, wkv2 = _q8pair(wkvf)
        wcf = np.float32(Wc[g * LH * D:(g + 1) * LH * D]) * SC  # [1024, C]
        wc1, wc2 = _q8pair(wcf)

        def arr_wc(a):  # [1024, C] -> [128(d), 4(lhp), 2, C]
            return np.ascontiguousarray(a.reshape(LH // 2, 2, 128, C).transpose(2, 0, 1, 3))

        in_maps.append(
            {
                "x1": x1s[b],
                "x2": x2s[b],
                "wq1": arr_w(wq1),
                "wq2": arr_w(wq2),
                "wkv1": arr_w(wkv1),
                "wkv2": arr_w(wkv2),
                "wc1": arr_wc(wc1),
                "wc2": arr_wc(wc2),
                "cs": cs,
                "tri": tri,
                "ident": ident,
            }
        )
    return in_maps


def kernel(x, freqs_cis, Wq, Wk, Wv, Wc):
    global LAST_RESULTS, _CACHED_NC
    if _CACHED_NC is None:
        _CACHED_NC = _build_module()
    nc = _CACHED_NC
    in_maps = _prep_inputs(x, freqs_cis, Wq, Wk, Wv, Wc)
    res = run_bass_kernel_spmd(nc, in_maps, core_ids=list(range(8)))
    LAST_RESULTS = res
    out = np.empty((B, T, C), dtype=np.float32)
    for b in range(B):
        acc = res.results[2 * b]["outT"] + res.results[2 * b + 1]["outT"]
        # SV: v-projection prescale, cancelled here instead of on-device
        out[b] = acc.T / (SC * SV)
    return out


# revision 56
# speedup vs baseline: 1.0410x; 1.0062x over previous
"""Causal GQA self-attention (B=4,T=2048,C=2048,H=16,HKV=4,D=128) on 8 trn2 cores.

Sharding: core c -> (batch b = c//2, kv-group g = c%2). Each core computes the
attention output for its batch restricted to its 8 query heads (2 kv heads),
then the partial output projection against the matching 1024 rows of Wc.
Host sums the two partial outputs per batch and descales. No collectives.

Speed plan vs bf16 baseline:
- QKV projection and output projection run as fp8e4m3 DoubleRow matmuls with a
  3-term error-compensated split (a@b ~= a1@b1 + a2@b1 + a1@b2, fp32 PSUM),
  contracting 256 rows/instr at 0.5 cycles/row.
- Attention internals (q/k/v/p) are fp16. Softmax skips max subtraction
  (rmsnormed q,k bound |scores| <= sqrt(D)); exp is biased by e^-3 to keep
  fp16 headroom; the bias cancels in the normalization.
- rmsnorm scale (and the softmax 1/sqrt(D) for q) is folded into the
  [token -> d,t] transpose by multiplying with diag(rstd) on the PE.
- Causal structure exploited at 128-key-block granularity: scores / exp / PV
  / row-sum accumulation all narrowed to the valid query suffix; one shared
  [128,128] upper-triangular mask handles the diagonal blocks.
- Row sums via an all-ones [128,128] stationary matmul producing the sum
  replicated across partitions (no separate broadcast matmul / copy).
- Output projection is interleaved into the attention loop (t4-major) so its
  dense fp8 matmuls fill the PE gaps left by the exp/mask dependency chain.
- Weight DMAs are chunked per contraction pair in consumption order so the
  first projection matmul starts ~2us after launch.
"""

import math
import sys

import numpy as np

sys.path.insert(0, "/opt/trn_rl_repo")

import ml_dtypes

import concourse.bass as bass
import concourse.bass_isa as bass_isa
import concourse.mybir as mybir
import concourse.tile as tile
from concourse import bacc
from concourse.bass_utils import run_bass_kernel_spmd

F8 = mybir.dt.float8e4
F16 = mybir.dt.float16
FP32 = mybir.dt.float32
NPF8 = ml_dtypes.float8_e4m3
NPF16 = np.float16

B, T, C = 4, 2048, 2048
H, HKV, D = 16, 4, 128
LH, LKV = 8, 2           # local q heads / kv heads per core
NTT = T // 128           # token tiles
NCP = C // 256           # contraction pairs (256 rows per DoubleRow matmul)
NQT = T // 512           # query tiles of 512
NH = LH + LKV            # heads normed/roped per token tile
NCT = C // 128           # output row tiles (phase 3)
EPS = 1e-6
SM = 1.0 / math.sqrt(D)  # softmax scale (folded into q's rstd)
SW = 64.0                # Wq/Wk fp8 prescale
SV = 32.0                # Wv fp8 prescale (descaled via ones rowsum matmul)
SC = 64.0                # Wc fp8 prescale (descaled on host)
EXPB = -3.0              # exp bias, cancels in normalization

ACT = mybir.ActivationFunctionType
DR = mybir.MatmulPerfMode.DoubleRow

USE_GPSIMD_STATS = False  # square on Pool engine instead of DVE

LAST_RESULTS = None       # BassKernelResults of the most recent run (test.py)
_CACHED_NC = None


def _build_module():
    nc = bacc.Bacc("TRN2", target_bir_lowering=False, debug=False, num_devices=8)

    x1_d = nc.dram_tensor("x1", [NTT, 128, NCP, 2, 128], F8, kind="ExternalInput").ap()
    x2_d = nc.dram_tensor("x2", [NTT, 128, NCP, 2, 128], F8, kind="ExternalInput").ap()
    wq1_d = nc.dram_tensor("wq1", [128, NCP, 2, LH * D], F8, kind="ExternalInput").ap()
    wq2_d = nc.dram_tensor("wq2", [128, NCP, 2, LH * D], F8, kind="ExternalInput").ap()
    wkv1_d = nc.dram_tensor("wkv1", [128, NCP, 2, 512], F8, kind="ExternalInput").ap()
    wkv2_d = nc.dram_tensor("wkv2", [128, NCP, 2, 512], F8, kind="ExternalInput").ap()
    wc1_d = nc.dram_tensor("wc1", [128, LH // 2, 2, C], F8, kind="ExternalInput").ap()
    wc2_d = nc.dram_tensor("wc2", [128, LH // 2, 2, C], F8, kind="ExternalInput").ap()
    cs_d = nc.dram_tensor("cs", [NTT, 128, 128], F16, kind="ExternalInput").ap()
    tri_d = nc.dram_tensor("tri", [128, 128], F16, kind="ExternalInput").ap()
    id_d = nc.dram_tensor("ident", [128, 128], F16, kind="ExternalInput").ap()
    out_d = nc.dram_tensor("outT", [C, T], FP32, kind="ExternalOutput").ap()

    with tile.TileContext(nc) as tc:
        consts = tc.alloc_tile_pool(name="consts", bufs=1)
        persist = tc.alloc_tile_pool(name="persist", bufs=1)

        eye_s = consts.tile([128, 128], F16)
        nc.sync.dma_start(out=eye_s, in_=id_d)
        tri_s = consts.tile([128, 128], F16)
        nc.sync.dma_start(out=tri_s, in_=tri_d)
        ones_m = consts.tile([128, 128], F16)
        nc.vector.memset(ones_m, 1.0)
        bias_q = consts.tile([128, 1], FP32)
        nc.vector.memset(bias_q, SW * SW * EPS / (SM * SM))
        bias_k = consts.tile([128, 1], FP32)
        nc.vector.memset(bias_k, SW * SW * EPS)
        bias_e = consts.tile([128, 1], FP32)
        nc.vector.memset(bias_e, EXPB)

        # persistent activations
        qt_all = persist.tile([128, LH, T], F16)      # Q^T per head [d, t]
        kt_all = persist.tile([128, LKV, T], F16)     # K^T per kv head [d, t]
        v_all = persist.tile([128, NTT, LKV * D], F16)  # V per t-tile [t, d]
        y1_all = persist.tile([128, LH, T], F8)       # attn out y^T hi
        y2_all = persist.tile([128, LH, T], F8)       # attn out y^T lo residual

        # ---------------- phase 1: projections + rmsnorm + rope + transpose
        with (
            tc.tile_pool(name="wpool", bufs=1) as wpool,
            tc.tile_pool(name="xpool", bufs=4) as xpool,
            tc.tile_pool(name="cspool", bufs=4) as cspool,
            tc.tile_pool(name="stage", bufs=2) as stage,
            tc.tile_pool(name="dpool", bufs=3) as dpool,
            tc.tile_pool(name="stats", bufs=3) as stats,
            tc.tile_pool(name="ps1", bufs=2, space="PSUM") as ps1,
            tc.tile_pool(name="pstp", bufs=2, space="PSUM") as pstp,
        ):
            wq1_s = wpool.tile([128, NCP, 2, LH * D], F8)
            wkv1_s = wpool.tile([128, NCP, 2, 512], F8)
            wq2_s = wpool.tile([128, NCP, 2, LH * D], F8)
            wkv2_s = wpool.tile([128, NCP, 2, 512], F8)

            def fetch_x(tt):
                xt1 = xpool.tile([128, NCP, 2, 128], F8, tag="x1")
                nc.sync.dma_start(out=xt1, in_=x1_d[tt])
                xt2 = xpool.tile([128, NCP, 2, 128], F8, tag="x2")
                nc.sync.dma_start(out=xt2, in_=x2_d[tt])
                cs_s = cspool.tile([128, 2, 64], F16)
                nc.sync.dma_start(
                    out=cs_s, in_=cs_d[tt].rearrange("p (a b) -> p a b", a=2)
                )
                return xt1, xt2, cs_s

            # x tiles 0/1 queue ahead of the weight stream so the first
            # matmuls only wait for ctp=0's weight slices; 2/3 interleave
            # into the weight stream ahead of when their tiles run
            xfetch = [fetch_x(0), fetch_x(1)]
            for c in range(NCP):
                nc.sync.dma_start(out=wq1_s[:, c], in_=wq1_d[:, c])
                nc.sync.dma_start(out=wkv1_s[:, c], in_=wkv1_d[:, c])
                nc.sync.dma_start(out=wq2_s[:, c], in_=wq2_d[:, c])
                nc.sync.dma_start(out=wkv2_s[:, c], in_=wkv2_d[:, c])
                if c in (3, 5):
                    xfetch.append(fetch_x(2 + (c - 3) // 2))

            # transposes of tile t emit after tile t+1's projection matmuls
            # so the PE never waits on tile t's rope/rstd chain
            pending_tp = []

            def emit_tp():
                while pending_tp:
                    pending_tp.pop(0)()

            for tt in range(NTT):
                xt1, xt2, cs_s = xfetch.pop(0)

                qkv_ps = ps1.tile([128, 1536], FP32)
                terms = [(xt1, wq1_s, wkv1_s), (xt2, wq1_s, wkv1_s), (xt1, wq2_s, wkv2_s)]
                for ctp in range(NCP):
                    for ti, (xs, wq, wkv) in enumerate(terms):
                        st = ctp == 0 and ti == 0
                        sp = ctp == NCP - 1 and ti == len(terms) - 1
                        nc.tensor.matmul(
                            qkv_ps[:, 0:512], lhsT=xs[:, ctp],
                            rhs=wq[:, ctp, :, 0:512], start=st, stop=sp,
                            perf_mode=DR,
                        )
                        nc.tensor.matmul(
                            qkv_ps[:, 512:1024], lhsT=xs[:, ctp],
                            rhs=wq[:, ctp, :, 512:1024], start=st, stop=sp,
                            perf_mode=DR,
                        )
                        nc.tensor.matmul(
                            qkv_ps[:, 1024:1536], lhsT=xs[:, ctp],
                            rhs=wkv[:, ctp], start=st, stop=sp,
                            perf_mode=DR,
                        )

                emit_tp()

                # V (cols 1280:1536) straight out, fp16, still x SV
                nc.scalar.copy(v_all[:, tt], qkv_ps[:, 1280:1536])
                # q/k raw to fp16 SBUF (values x SW)
                raw = stage.tile([128, NH, 128], F16, tag="raw")
                nc.scalar.copy(
                    raw, qkv_ps[:, 0:1280].rearrange("p (h d) -> p h d", h=NH)
                )

                # rmsnorm stats: square on GPSIMD (idle), reduce on DVE.
                sq_eng = nc.gpsimd if USE_GPSIMD_STATS else nc.vector
                sq = stage.tile([128, NH, 128], FP32, tag="sq")
                sq_eng.tensor_mul(sq, raw, raw)
                ssq = stats.tile([128, NH], FP32)
                nc.vector.tensor_reduce(
                    out=ssq, in_=sq, axis=mybir.AxisListType.X, op=mybir.AluOpType.add
                )
                # rstd_q = SM / (SW*sqrt(ms+eps)); rstd_k = 1 / (SW*sqrt(ms+eps))
                srt = stats.tile([128, NH], FP32)
                nc.scalar.activation(
                    srt[:, 0:LH], ssq[:, 0:LH], ACT.Sqrt,
                    scale=1.0 / (D * SM * SM), bias=bias_q,
                )
                nc.scalar.activation(
                    srt[:, LH:NH], ssq[:, LH:NH], ACT.Sqrt,
                    scale=1.0 / D, bias=bias_k,
                )
                rstd = stats.tile([128, NH], FP32)
                nc.vector.reciprocal(out=rstd, in_=srt)

                # rope on raw (unscaled; rstd applied during transpose).
                # W columns are host-permuted even-dims-first so the rotation
                # halves are contiguous (keeps DVE 16-bit 2x mode)
                cos_b = cs_s[:, 0:1, :].broadcast_to([128, NH, 64])
                sin_b = cs_s[:, 1:2, :].broadcast_to([128, NH, 64])
                q_e, q_o = raw[:, :, 0:64], raw[:, :, 64:128]
                rp = stage.tile([128, NH, 128], F16, tag="rp")
                r_e, r_o = rp[:, :, 0:64], rp[:, :, 64:128]
                t1 = stage.tile([128, NH, 64], F16, tag="t1")
                t2 = stage.tile([128, NH, 64], F16, tag="t2")
                nc.vector.tensor_mul(t1, q_e, cos_b)
                nc.vector.tensor_mul(t2, q_o, sin_b)
                nc.vector.tensor_sub(r_e, t1, t2)
                t3 = stage.tile([128, NH, 64], F16, tag="t3")
                t4 = stage.tile([128, NH, 64], F16, tag="t4")
                nc.vector.tensor_mul(t3, q_e, sin_b)
                nc.vector.tensor_mul(t4, q_o, cos_b)
                nc.vector.tensor_add(r_o, t3, t4)

                # transpose to [d, t] with rstd folded in via diag matmul;
                # deferred so it emits after the next tile's projections
                def transpose_tile(tt=tt, rp=rp, rstd=rstd):
                    for g0, g1 in ((0, 4), (4, 8), (8, 10)):
                        n = g1 - g0
                        tp = pstp.tile([128, 512], FP32, tag="tp")
                        for i in range(g0, g1):
                            dg = dpool.tile([128, 128], F16)
                            nc.vector.tensor_scalar_mul(dg, eye_s, rstd[:, i:i + 1])
                            nc.tensor.matmul(
                                tp[:, (i - g0) * 128:(i - g0 + 1) * 128],
                                lhsT=rp[:, i], rhs=dg, start=True, stop=True,
                            )
                        view = tp[:, 0:n * 128].rearrange("p (h d) -> p h d", h=n)
                        if g0 < 8:
                            dst = qt_all[:, g0:g1, tt * 128:(tt + 1) * 128]
                        else:
                            dst = kt_all[:, g0 - 8:g1 - 8, tt * 128:(tt + 1) * 128]
                        nc.scalar.copy(dst, view)
                pending_tp.append(transpose_tile)

                if tt + 4 < NTT:
                    xfetch.append(fetch_x(tt + 4))

            emit_tp()

        # ---------------- phase 2: attention, with phase-3 chunks interleaved
        with tc.tile_pool(name="wcpool", bufs=1) as wcpool:
            # Wc (fp8 pair) loads overlap attention; used by phase-3 chunks
            wc1_s = wcpool.tile([128, LH // 2, 2, C], F8)
            nc.sync.dma_start(out=wc1_s, in_=wc1_d)
            wc2_s = wcpool.tile([128, LH // 2, 2, C], F8)
            nc.sync.dma_start(out=wc2_s, in_=wc2_d)

            with (
                tc.tile_pool(name="ppool", bufs=8) as ppool,
                tc.tile_pool(name="accp", bufs=2) as accp,
                tc.tile_pool(name="rcpool", bufs=2) as rcpool,
                tc.tile_pool(name="ypool", bufs=6) as ypool,
                tc.tile_pool(name="outst", bufs=3) as outst,
                tc.tile_pool(name="rspool", bufs=2) as rspool,
                tc.tile_pool(name="pss", bufs=2, space="PSUM") as pss,
                tc.tile_pool(name="pso", bufs=2, space="PSUM") as pso,
                tc.tile_pool(name="psop", bufs=2, space="PSUM") as psop,
            ):
                p3_terms = [(y1_all, wc1_s), (y2_all, wc1_s), (y1_all, wc2_s)]

                # finishers (psum drain + DMA) run one emission point later so
                # they never head-block the ACT/DVE queues behind a PE wait
                p3_finishers = []

                def emit_p3_chunk(t4, ct):
                    """One output-projection chunk: 12 DR matmuls; the psum
                    drain + output DMA are deferred to the next chunk."""
                    # y stores for this t4 must be emitted before reading y
                    while tail_finishers:
                        tail_finishers.pop(0)()
                    while p3_finishers:
                        p3_finishers.pop(0)()
                    op_ps = psop.tile([128, 512], FP32, tag="op")
                    for ti, (ya, wc) in enumerate(p3_terms):
                        for lhp in range(LH // 2):
                            st = ti == 0 and lhp == 0
                            sp = ti == len(p3_terms) - 1 and lhp == LH // 2 - 1
                            nc.tensor.matmul(
                                op_ps,
                                lhsT=wc[:, lhp, :, ct * 128:(ct + 1) * 128],
                                rhs=ya[:, 2 * lhp:2 * lhp + 2, t4 * 512:(t4 + 1) * 512],
                                start=st, stop=sp, perf_mode=DR,
                            )

                    def finish(t4=t4, ct=ct, op_ps=op_ps):
                        ost = outst.tile([128, 512], FP32)
                        nc.scalar.copy(ost, op_ps)
                        nc.sync.dma_start(
                            out=out_d[ct * 128:(ct + 1) * 128, t4 * 512:(t4 + 1) * 512],
                            in_=ost,
                        )
                    p3_finishers.append(finish)

                # pending phase-3 work: chunks for t4 become ready once all lh
                # of qt=t4 have run; trickled into later iterations' gaps.
                p3_pending = []
                done_lh = [0] * NQT

                # deferred per-iteration tail (rowsum..yt), emitted after the
                # next iteration's first score pair to keep the PE queue fed;
                # the y1/y2 stores defer one step further (not latency-bound)
                pending_tail = []
                tail_finishers = []

                def emit_tail():
                    while pending_tail:
                        fin = pending_tail.pop(0)()
                        tail_finishers.append(fin)
                    # y1/y2 stores lag a few iterations: they are only read
                    # by p3 chunks (which force-drain) so they can vacate the
                    # tight blocks entirely
                    while len(tail_finishers) > 3:
                        tail_finishers.pop(0)()

                # 3-way interleave of qt 3/1/0 balances PE vs ACT load and
                # hides each iteration's softmax-tail latency under the other
                # two; qt=2 runs last, its gaps filled by the 48 output-
                # projection chunks that became ready (t4 in {3,1,0})
                schedule = [(q, l) for l in range(LH) for q in (1, 0)]
                schedule += [(2, l) for l in range(LH)]
                schedule += [(3, l) for l in range(LH)]

                class It:
                    """Per-(query-tile, head) iteration state."""

                    def __init__(self, qt, lh):
                        self.qt, self.lh = qt, lh
                        self.lkv = lh // (H // HKV)
                        self.npair = 2 * qt + 2
                        self.nkb = 2 * self.npair
                        self.q0 = qt * 512
                        self.o_ps = None
                        self.pacc = None
                        self.pts = [None] * self.npair

                def emit_scores(it, pi):
                    qt = it.qt
                    s_ps = pss.tile([128, 1024], FP32)
                    pt = ppool.tile([128, 2, 512], F16)
                    for e in range(2):
                        kb = 2 * pi + e
                        j = kb - 4 * qt
                        vs = 128 * j if j > 0 else 0
                        nc.tensor.matmul(
                            s_ps[:, e * 512 + vs:(e + 1) * 512],
                            lhsT=kt_all[:, it.lkv, kb * 128:(kb + 1) * 128],
                            rhs=qt_all[:, it.lh, it.q0 + vs:it.q0 + 512],
                            start=True, stop=True,
                        )
                    if pi < 2 * qt:
                        # off-diagonal pair: one full-width exp
                        nc.scalar.activation(
                            pt.rearrange("p a b -> p (a b)"), s_ps, ACT.Exp,
                            bias=bias_e, scale=1.0,
                        )
                    else:
                        # diagonal pair: narrowed per-block exp + tri mask
                        for e in range(2):
                            kb = 2 * pi + e
                            j = kb - 4 * qt
                            vs = 128 * j if j > 0 else 0
                            nc.scalar.activation(
                                pt[:, e, vs:512],
                                s_ps[:, e * 512 + vs:(e + 1) * 512],
                                ACT.Exp, bias=bias_e, scale=1.0,
                            )
                            nc.vector.tensor_mul(
                                pt[:, e, vs:vs + 128], pt[:, e, vs:vs + 128],
                                tri_s,
                            )
                    it.pts[pi] = pt

                def emit_pv_acc(it, pi):
                    qt = it.qt
                    if pi == 0:
                        it.o_ps = pso.tile([128, 512], FP32)
                        it.pacc = accp.tile([128, 512], F16)
                    pt = it.pts[pi]
                    it.pts[pi] = None
                    for e in range(2):
                        kb = 2 * pi + e
                        j = kb - 4 * qt
                        vs = 128 * j if j > 0 else 0
                        nc.tensor.matmul(
                            it.o_ps[:, vs:512],
                            lhsT=v_all[:, kb, it.lkv * D:(it.lkv + 1) * D],
                            rhs=pt[:, e, vs:512],
                            start=(kb == 0), stop=(kb == it.nkb - 1),
                        )
                    # row-sum accumulator, both key blocks folded into one
                    # [128, 512] lane (valid-region ops only)
                    for e in range(2):
                        kb = 2 * pi + e
                        j = kb - 4 * qt
                        vs = 128 * j if j > 0 else 0
                        if pi == 0 and e == 0:
                            nc.vector.tensor_copy(out=it.pacc, in_=pt[:, 0])
                        elif vs == 0:
                            nc.vector.tensor_add(it.pacc, it.pacc, pt[:, e])
                        else:
                            nc.vector.tensor_add(
                                it.pacc[:, vs:512], it.pacc[:, vs:512],
                                pt[:, e, vs:512],
                            )

                def make_tail(it):
                    def tail():
                        # softmax denominator, summed over keys and replicated
                        # to all partitions. qt<=1 iterations run in a block
                        # where ACT/DVE/Pool are tight but the PE and the p3
                        # psum bank are idle -> all-ones matmul there; the
                        # rest use a Pool partition all-reduce.
                        if it.qt <= 1:
                            rs = psop.tile([128, 512], FP32, tag="op")
                            nc.tensor.matmul(
                                rs, lhsT=ones_m, rhs=it.pacc,
                                start=True, stop=True,
                            )
                        else:
                            rs = rspool.tile([128, 512], FP32, tag="rs")
                            nc.gpsimd.partition_all_reduce(
                                rs, it.pacc, channels=128,
                                reduce_op=bass_isa.ReduceOp.add,
                            )
                        bc_sb = rcpool.tile([128, 512], FP32)
                        with nc.allow_low_precision(
                            reason="softmax denominator reciprocal; fp32 data"
                        ):
                            nc.vector.reciprocal(out=bc_sb, in_=rs)
                        yt = ypool.tile([128, 512], F16)
                        nc.vector.tensor_mul(yt, it.o_ps, bc_sb)

                        def finish():
                            y1s = y1_all[:, it.lh, it.qt * 512:(it.qt + 1) * 512]
                            y2s = y2_all[:, it.lh, it.qt * 512:(it.qt + 1) * 512]
                            nc.gpsimd.tensor_copy(out=y1s, in_=yt)
                            nc.gpsimd.tensor_sub(y2s, yt, y1s)
                        return finish
                    return tail

                # globally software-pipelined pair stream: the scores+exp of
                # pair k+2 emit before the PV of pair k, across iteration
                # boundaries, so the PE never heads an exp it just requested
                iters = [It(qt, lh) for qt, lh in schedule]
                stream = [(it, pi) for it in iters for pi in range(it.npair)]
                for k in range(min(2, len(stream))):
                    emit_scores(*stream[k])
                for k, (it, pi) in enumerate(stream):
                    if k + 2 < len(stream):
                        emit_scores(*stream[k + 2])
                    if pi == 0:
                        emit_tail()
                        for _ in range(2):
                            if p3_pending:
                                emit_p3_chunk(*p3_pending.pop(0))
                    emit_pv_acc(it, pi)
                    if pi == it.npair - 1:
                        pending_tail.append(make_tail(it))
                        done_lh[it.qt] += 1
                        if done_lh[it.qt] == LH:
                            # all heads of this qt done -> p3 tile ready
                            p3_pending.extend((it.qt, ct) for ct in range(NCT))
                    elif pi < 2 and p3_pending:
                        emit_p3_chunk(*p3_pending.pop(0))

                emit_tail()
                while tail_finishers:
                    tail_finishers.pop(0)()
                for t4, ct in p3_pending:
                    emit_p3_chunk(t4, ct)
                while p3_finishers:
                    p3_finishers.pop(0)()

        persist.release()
        consts.release()

    nc.compile()
    return nc


def _q8pair(a):
    """fp8 hi/lo split of a float32 array."""
    a1 = a.astype(NPF8)
    a2 = (a - a1.astype(np.float32)).astype(NPF8)
    return a1, a2


def _prep_inputs(x, freqs_cis, Wq, Wk, Wv, Wc):
    """Host-side shard + layout + quantization prep; returns 8 input maps."""
    x = np.asarray(x, dtype=np.float32)
    freqs_cis = np.asarray(freqs_cis, dtype=np.float32)

    x1, x2 = _q8pair(x)

    def arr_x(a):  # [T, C] -> [NTT, 128(c-part), NCP, 2, 128(tok)]
        t = a.T.reshape(NCP, 2, 128, NTT, 128).transpose(3, 2, 0, 1, 4)
        return np.ascontiguousarray(t)

    x1s = [arr_x(x1[b]) for b in range(B)]
    x2s = [arr_x(x2[b]) for b in range(B)]

    cs = np.concatenate([freqs_cis[:, :, 0], freqs_cis[:, :, 1]], axis=1)  # [T,128]
    cs = np.ascontiguousarray(cs.reshape(NTT, 128, 128)).astype(NPF16)

    # upper-triangular inclusive mask: tri[r, j] = 1 if j >= r
    tri = np.triu(np.ones((128, 128), dtype=NPF16))

    ident = np.eye(128, dtype=NPF16)

    def arr_w(a):  # [C, n] -> [128, NCP, 2, n]
        return np.ascontiguousarray(a.reshape(NCP, 2, 128, a.shape[1]).transpose(2, 0, 1, 3))

    # permute each head's D dims even-first/odd-last for contiguous rope;
    # scores are invariant since q and k share the permutation
    dperm = np.concatenate([np.arange(0, D, 2), np.arange(1, D, 2)])

    def permute_heads(w, nh):
        return np.ascontiguousarray(
            w.reshape(C, nh, D)[:, :, dperm].reshape(C, nh * D)
        )

    in_maps = []
    for core in range(8):
        b, g = divmod(core, 2)
        wqf = permute_heads(
            np.float32(Wq[:, g * LH * D:(g + 1) * LH * D]), LH
        ) * SW
        wq1, wq2 = _q8pair(wqf)
        wkvf = np.concatenate(
            [
                permute_heads(
                    np.float32(Wk[:, g * LKV * D:(g + 1) * LKV * D]), LKV
                ) * SW,
                np.float32(Wv[:, g * LKV * D:(g + 1) * LKV * D]) * SV,
            ],
            axis=1,
        )
        wkv1, wkv2 = _q8pair(wkvf)
        wcf = np.float32(Wc[g * LH * D:(g + 1) * LH * D]) * SC  # [1024, C]
        wc1, wc2 = _q8pair(wcf)

        def arr_wc(a):  # [1024, C] -> [128(d), 4(lhp), 2, C]
            return np.ascontiguousarray(a.reshape(LH // 2, 2, 128, C).transpose(2, 0, 1, 3))

        in_maps.append(
            {
                "x1": x1s[b],
                "x2": x2s[b],
                "wq1": arr_w(wq1),
                "wq2": arr_w(wq2),
                "wkv1": arr_w(wkv1),
                "wkv2": arr_w(wkv2),
                "wc1": arr_wc(wc1),
                "wc2": arr_wc(wc2),
                "cs": cs,
                "tri": tri,
                "ident": ident,
            }
        )
    return in_maps


def kernel(x, freqs_cis, Wq, Wk, Wv, Wc):
    global LAST_RESULTS, _CACHED_NC
    if _CACHED_NC is None:
        _CACHED_NC = _build_module()
    nc = _CACHED_NC
    in_maps = _prep_inputs(x, freqs_cis, Wq, Wk, Wv, Wc)
    res = run_bass_kernel_spmd(nc, in_maps, core_ids=list(range(8)))
    LAST_RESULTS = res
    out = np.empty((B, T, C), dtype=np.float32)
    for b in range(B):
        acc = res.results[2 * b]["outT"] + res.results[2 * b + 1]["outT"]
        # SV: v-projection prescale, cancelled here instead of on-device
        out[b] = acc.T / (SC * SV)
    return out
